# revision 57
# baseline (speedup 1.0000x reference)
"""CrossAttention Trainium2 kernel.

Full-input contract: kernel(**inputs) takes the unsharded tensors
(x [32,1024,640], y [32,77,768], Wq,bq,Wk,bk,Wv,bv,Wo,bo) and returns
the full [32,1024,640] output.  Internally: data-parallel over batch
across 8 NeuronCores (4 batches per core), one shared SPMD Bass/Tile
kernel, no collectives.

Per-core dataflow (fp32 data; matmuls in float32r single-pass mode):
  x -> xT and y -> yT via TensorE transposes (fp32 has no DMA
  transpose); head dim padded 80->96 with zero weight columns so each
  per-head tile has its own partition range.
  KT = WkT yT (per head), V = y Wv.
  Per 512-wide q block, per head (Q projection interleaved so PE has
  fill work during the softmax chain):
    QT_h = WqT_h xT + bq       [96, 512]  (1/sqrt(D) folded into Wq)
    ST   = KT_h^T QT_h         [77, 512]  scores, transposed
    Ew   = exp(ST)             [77, 512]  ScalarE
    F    = ones^T Ew           [96, 512]  matmul row-broadcasts sums
    O    = V_h^T Ew            [96, 512]
    rcF  = exp(-ln F)          ScalarE (one pinned ACT table set)
    attnT_h = O * rcF          fused into the PSUM evacuation (DVE)
  out = attnT^T Wo_pad + bo    per 128-row q chunk.

Softmax needs no max subtraction: scores/sqrt(D) ~ N(0,1); max over
20M samples is ~6 sigma, far inside fp32 exp range.
"""

import os
import sys

import ml_dtypes
import numpy as np

BF16 = ml_dtypes.bfloat16

for _p in ("/opt/trn_rl_repo", os.path.expanduser("~/.axon_site/_ro/trn_rl_repo")):
    if os.path.isdir(_p) and _p not in sys.path:
        sys.path.insert(0, _p)
        break

# --- problem constants (hardcoded per contract) ---
B, SQ, SKV = 32, 1024, 77
E, C = 640, 768
H, D = 8, 80
DP = 96                # padded head dim (80 -> 96, 32-aligned)
EP = H * DP            # 768
N_CORES = 8
B_LOC = B // N_CORES   # 4
P = 128
QBLK = 512
SCALE = 1.0 / float(np.sqrt(D))

LAST_RESULTS = None  # BassKernelResults of the most recent run (for test.py)

_BUILT = None

# Packed-permuted layout for the QT/KT contraction rows: the padded 768
# head dims are permuted into 6 chunks of 128 so each head's 96 rows form
# a 64-row piece (base 0 or 64) plus 32-row pieces -- matmul operand base
# partitions must be 0/32/64.  SSEGS[h] lists (chunk, base, len).
_SSEGS = {
    0: [(0, 0, 64), (1, 0, 32)],
    1: [(0, 64, 64), (1, 32, 32)],
    2: [(1, 64, 64), (2, 0, 32)],
    3: [(2, 32, 32), (2, 64, 64)],
}
for _h in range(4):
    _SSEGS[_h + 4] = [(j + 3, s, l) for (j, s, l) in _SSEGS[_h]]

# _PERM[packed_row] = padded head-dim index (for permuting wq/wk columns)
_PERM = np.zeros(8 * 96, np.int64)
for _h, _segs in _SSEGS.items():
    _d = 96 * _h
    for (_j, _s, _l) in _segs:
        _PERM[128 * _j + _s : 128 * _j + _s + _l] = np.arange(_d, _d + _l)
        _d += _l
assert sorted(_PERM.tolist()) == list(range(768))


def _pad_cols(W):
    """[in, H*D] -> [in, H*DP], per-head zero-padded columns."""
    Wp = np.zeros((W.shape[0], EP), np.float32)
    for h in range(H):
        Wp[:, h * DP : h * DP + D] = W[:, h * D : (h + 1) * D]
    return Wp


def _pad_vec(b):
    bp = np.zeros((EP,), np.float32)
    for h in range(H):
        bp[h * DP : h * DP + D] = b[h * D : (h + 1) * D]
    return bp


def _pad_rows(W):
    """[H*D, out] -> [H*DP, out], per-head zero-padded rows."""
    Wp = np.zeros((EP, W.shape[1]), np.float32)
    for h in range(H):
        Wp[h * DP : h * DP + D] = W[h * D : (h + 1) * D]
    return Wp


def _build():
    """Build the SPMD Bass kernel once; returns (nc, input tensor names)."""
    import concourse.bass as bass
    import concourse.bacc as bacc
    import concourse.mybir as mybir
    import concourse.tile as tile
    from contextlib import ExitStack

    f32 = mybir.dt.float32
    f32r = mybir.dt.float32r
    bf16 = mybir.dt.bfloat16
    AF = mybir.ActivationFunctionType
    ALU = mybir.AluOpType

    import bass_rust as _bass_rust
    from concourse.hw_specs import get_activation_tables

    class _Bacc(bacc.Bacc):
        # All our ACT functions (Exp, Ln, Copy, Identity) live in the
        # natural_log_exp_and_others set.  The stock greedy table-load pass
        # thrashes between exp_and_others and natural_log (129 loads,
        # ~165us); blank every other set so each ACTIVATE resolves to the
        # one shared set (indices preserved for walrus).
        def insert_act_table_loads(self):
            has_activation = any(
                isinstance(i, mybir.InstActivation)
                for blk in self.main_func.blocks
                for i in blk.instructions
            )
            if not has_activation:
                return
            tables = [
                (name, funcs if name == "natural_log_exp_and_others" else set())
                for name, funcs in get_activation_tables(self.m.arch).items()
            ]
            _bass_rust.insert_act_table_loads(self, tables)

    nc = _Bacc("TRN2", target_bir_lowering=False, debug=False)

    x_d = nc.dram_tensor("x", [B_LOC, SQ, E], bf16, kind="ExternalInput").ap()
    y_d = nc.dram_tensor("y", [B_LOC, SKV, C], f32, kind="ExternalInput").ap()
    wq_d = nc.dram_tensor("wq", [E, EP], bf16, kind="ExternalInput").ap()
    # Per-key softmax bias sbias[k,h,b] = (y@Wk)_hk . bq_h: the bq term of
    # (k+bk).(q+bq); the bk term is constant per query and cancels in
    # softmax, so K and Q projections run bias-free (host computes sbias).
    sb_d = nc.dram_tensor("sbias", [SKV, H, B_LOC], f32, kind="ExternalInput").ap()
    wk_d = nc.dram_tensor("wk", [C, EP], f32, kind="ExternalInput").ap()
    wv_d = nc.dram_tensor("wv", [C, EP], f32, kind="ExternalInput").ap()
    bv_d = nc.dram_tensor("bv", [P, EP], f32, kind="ExternalInput").ap()
    wo_d = nc.dram_tensor("wo", [EP, E], f32, kind="ExternalInput").ap()
    bo_d = nc.dram_tensor("bo", [P, E], f32, kind="ExternalInput").ap()
    ones_d = nc.dram_tensor("ones", [SKV, DP], f32, kind="ExternalInput").ap()
    ident_d = nc.dram_tensor("ident", [P, P], f32, kind="ExternalInput").ap()
    identb_d = nc.dram_tensor("identb", [P, P], bf16, kind="ExternalInput").ap()
    out_d = nc.dram_tensor("out", [B_LOC, SQ, E], f32, kind="ExternalOutput").ap()

    EC = E // P   # 5 chunks over embed contraction
    CC = C // P   # 6 chunks over cross contraction
    NBLK = SQ // QBLK  # 2
    QC_PER_BLK = QBLK // P  # 4

    def r(ap):
        return ap.bitcast(f32r)

    with tile.TileContext(nc) as tc, ExitStack() as ctx:
        const = ctx.enter_context(tc.tile_pool(name="const", bufs=1))
        wpool = ctx.enter_context(tc.tile_pool(name="wts", bufs=1))
        kvpool = ctx.enter_context(tc.tile_pool(name="kv", bufs=1))
        xtpool = ctx.enter_context(tc.tile_pool(name="xt", bufs=3))
        # 8 PSUM banks: A2 (transposes+Q) B1 (S) F2 O2 out1.  Baseline had
        # F and O sharing one bank, serializing O's matmul behind ScalarE's
        # Ln every head.
        psA = ctx.enter_context(tc.tile_pool(name="psA", bufs=1, space="PSUM"))
        psB = ctx.enter_context(tc.tile_pool(name="psB", bufs=2, space="PSUM"))
        psF = ctx.enter_context(tc.tile_pool(name="psF", bufs=1, space="PSUM"))
        psO = ctx.enter_context(tc.tile_pool(name="psO", bufs=2, space="PSUM"))
        psout = ctx.enter_context(tc.tile_pool(name="psout", bufs=1, space="PSUM"))

        # ---- constants ----
        ident = const.tile([P, P], f32)
        nc.sync.dma_start(ident[:], ident_d)
        identb = const.tile([P, P], bf16)
        nc.sync.dma_start(identb[:], identb_d)
        ones_t = const.tile([SKV, DP], f32r)
        nc.sync.dma_start(ones_t[:], ones_d.bitcast(f32r))

        def phase_x(b):
            """DMA-transpose x[b] (bf16) straight into xT chunks -- no PE
            transposes, no PSUM, half the HBM traffic of fp32.  Only safe
            with long lead time (transposing-DMA completion is slow); batch
            0 uses phase_x0 instead."""
            xt = xtpool.tile([P, EC, SQ], bf16)
            for c in range(EC):
                nc.sync.dma_start_transpose(
                    xt[:, c, :], x_d[b, :, c * P : (c + 1) * P]
                )
            return xt

        def phase_x0(x0):
            """Batch 0 xT via PE transposes of a regular (tracked) DMA."""
            xt = xtpool.tile([P, EC, SQ], bf16)
            for c in range(EC):
                for g in range(2):
                    ps4 = psA.tile([P, 4, P], bf16, tag="q")
                    for k in range(4):
                        nc.tensor.transpose(
                            ps4[:, k, :], x0[:, 4 * g + k, c * P : (c + 1) * P],
                            identb[:],
                        )
                    if g == 0:
                        nc.scalar.copy(xt[:, c, 0:QBLK], ps4[:])
                    else:
                        nc.vector.tensor_copy(xt[:, c, QBLK:SQ], ps4[:])
            return xt

        # Weights stream per contraction chunk on their own HWDGE queues so
        # the K/V phase and the first Q projection never wait on one deep
        # queue: wk->scalar, wv->gpsimd, wq->tensor, wo->vector(late).
        # Chunk-wise DMAs let each matmul start as soon as its slice lands.
        kvw_ctx = ExitStack()
        kvwpool = kvw_ctx.enter_context(tc.tile_pool(name="kvw", bufs=1))
        y_all = kvwpool.tile([SKV, B_LOC, C], f32)
        for b in range(B_LOC):
            nc.scalar.dma_start(y_all[:, b], y_d[b])
        wk_r = wk_d.rearrange("(c p) f -> p c f", p=P).bitcast(f32r)
        wk_s = kvwpool.tile([P, CC, EP], f32r)
        for c in range(CC):
            nc.scalar.dma_start(wk_s[:, c], wk_r[:, c])
        wv_r = wv_d.rearrange("(c p) f -> p c f", p=P).bitcast(f32r)
        wv_s = kvwpool.tile([P, CC, EP], f32r)
        for c in range(CC):
            nc.gpsimd.dma_start(wv_s[:, c], wv_r[:, c])
        sb_s = const.tile([SKV, H, B_LOC], f32)
        nc.scalar.dma_start(sb_s[:], sb_d)
        bv_b = const.tile([P, EP], f32)
        nc.gpsimd.dma_start(bv_b[:], bv_d)

        wo_r = wo_d.rearrange("(h d) f -> d h f", d=DP).bitcast(f32r)
        wo_s = wpool.tile([DP, H, E], f32r)
        for h in range(H):
            nc.scalar.dma_start(wo_s[:, h], wo_r[:, h])
        bo_b = const.tile([P, E], f32)
        nc.scalar.dma_start(bo_b[:], bo_d)

        # x(b=0) goes ahead of wq on the sync queue; wq is not needed until
        # the first Q projection (~KV phase end).
        x0 = kvwpool.tile([P, SQ // P, E], bf16)
        nc.sync.dma_start(x0[:], x_d[0].rearrange("(q p) e -> p q e", p=P))
        wq_r = wq_d.rearrange("(c p) f -> p c f", p=P)
        wq_s = wpool.tile([P, EC, EP], bf16)
        for c in range(EC):
            nc.sync.dma_start(wq_s[:, c], wq_r[:, c])
        xt_cur = phase_x0(x0)
        # batch 1's transposing DMAs get the whole K/V phase of lead time
        xt_b1 = phase_x(1)

        # ---- y -> yT, K/V projections for all local batches ----
        yt = kvpool.tile([P, CC, B_LOC, SKV], f32r)
        for b in range(B_LOC):
            y_tile = y_all[:, b]
            for c0 in range(0, CC, 3):
                ps3 = psA.tile([P, 3, SKV], f32, tag="q")
                for c in range(3):
                    nc.tensor.transpose(
                        ps3[:, c, :],
                        y_tile[:, (c0 + c) * P : (c0 + c + 1) * P],
                        ident[:SKV, :SKV],
                    )
                nc.scalar.copy(yt[:, c0 : c0 + 3, b, :], ps3[:])

        # K projection over full 128-wide packed output chunks (36 matmuls
        # instead of 48), bias-free, evacuated as plain chunk copies.  The
        # attention consumes kt/qt in packed layout via 32/64-aligned
        # per-head partition segments (SSEGS).
        NJ = EP // P  # 6
        # SSEGS[h]: (chunk j, partition offset, len) covering head h's 96
        # padded rows in the packed-permuted [768] layout (see _PERM).
        # Matmul operand base partitions must be 0/32/64, so heads are
        # permuted into 64+32 pieces that never start at partition 96.
        SSEGS = dict(_SSEGS)

        kt_s = kvpool.tile([P, NJ, B_LOC, SKV], f32r)
        for j in range(NJ):
            ps_k = psA.tile([P, B_LOC, SKV], f32, tag="q")
            for c in range(CC):
                nc.tensor.matmul(
                    ps_k[:],
                    r(wk_s[:, c, j * P : (j + 1) * P]),
                    yt[:, c],
                    start=(c == 0),
                    stop=(c == CC - 1),
                )
            nc.scalar.copy(kt_s[:, j], ps_k[:])

        v_s = kvpool.tile([SKV, B_LOC, EP], f32r)
        for b in range(B_LOC):
            for n in range(2):  # EP = 2 x 384
                ps_v = psO.tile([SKV, 384], f32, tag="o")
                for c in range(CC):
                    nc.tensor.matmul(
                        ps_v[:],
                        yt[:, c, b, :],
                        r(wv_s[:, c, n * 384 : (n + 1) * 384]),
                        start=(c == 0),
                        stop=(c == CC - 1),
                    )
                nc.vector.tensor_tensor(
                    v_s[:, b, n * 384 : (n + 1) * 384],
                    ps_v[:],
                    bv_b[:SKV, n * 384 : (n + 1) * 384],
                    ALU.add,
                )

        kvw_ctx.close()

        qpool = ctx.enter_context(tc.tile_pool(name="q", bufs=2))
        spool = ctx.enter_context(tc.tile_pool(name="s", bufs=3))
        apool = ctx.enter_context(tc.tile_pool(name="attn", bufs=1))
        opool = ctx.enter_context(tc.tile_pool(name="ost", bufs=4))

        def qproj_chunk(xt, qs, qt, j):
            """Packed Q projection: rows 128j..128j+128 of the padded-[768]
            QT (full 128-wide stationary, 5 matmuls), bias-free, evacuated
            as one plain chunk copy on DVE."""
            ps_q = psA.tile([P, QBLK], f32, tag="q")
            for c in range(EC):
                nc.tensor.matmul(
                    ps_q[:], wq_s[:, c, j * P : (j + 1) * P], xt[:, c, qs],
                    start=(c == 0), stop=(c == EC - 1),
                )
            nc.vector.tensor_copy(qt[:, j], ps_q[:])

        # ---- main loop: flat (batch, block) steps; Q projection of step
        # i+1 is interleaved into step i's head chain (between S and F) so
        # PE streams packed Q chunks while ScalarE runs exp/ln.
        steps = [(b, blk) for b in range(B_LOC) for blk in range(NBLK)]
        xts = {0: xt_cur, 1: xt_b1}
        # Q-proj chunks are split: a step's own chunks 3-5 are emitted
        # inside its own head loop (heads 0-2; first consumer is head 4),
        # the next step's chunks 0-2 inside heads 4-6.  Every step --
        # including the last -- then has PE fill work during the ScalarE
        # softmax chain, which also keeps the HAM activity window warm.
        qt_cur = qpool.tile([P, NJ, QBLK], f32r, tag="qt")
        for j in range(3):
            qproj_chunk(xts[0], slice(0, QBLK), qt_cur, j)

        for si, (b, blk) in enumerate(steps):
            qs = slice(blk * QBLK, (blk + 1) * QBLK)
            qt = qt_cur
            nxt = steps[si + 1] if si + 1 < len(steps) else None
            if nxt is not None:
                qt_nxt = qpool.tile([P, NJ, QBLK], f32r, tag="qt")
                nqs = slice(nxt[1] * QBLK, (nxt[1] + 1) * QBLK)
            attn = apool.tile([DP, H, QBLK], f32r)
            for h in range(H):
                ps_s = psB.tile([SKV, QBLK], f32, tag="s")
                segs = SSEGS[h]
                for i, (j, s0, ln_) in enumerate(segs):
                    nc.tensor.matmul(
                        ps_s[:], kt_s[s0 : s0 + ln_, j, b, :],
                        qt[s0 : s0 + ln_, j], start=(i == 0),
                        stop=(i == len(segs) - 1),
                    )
                ew = spool.tile([SKV, QBLK], f32r, tag="ew")
                nc.scalar.activation(
                    ew[:], ps_s[:], AF.Exp, bias=sb_s[:, h, b : b + 1]
                )
                # Q-proj fill between S and F while ScalarE runs exp
                if h < 3:
                    qproj_chunk(xts[b], qs, qt, h + 3)
                elif 4 <= h < 7 and nxt is not None:
                    qproj_chunk(xts[nxt[0]], nqs, qt_nxt, h - 4)
                ps_f = psF.tile([DP, QBLK], f32, tag="f")
                nc.tensor.matmul(
                    ps_f[:], ones_t[:], ew[:], start=True, stop=True
                )
                ps_o = psO.tile([DP, QBLK], f32, tag="o")
                nc.tensor.matmul(
                    ps_o[:], r(v_s[:, b, h * DP : (h + 1) * DP]), ew[:],
                    start=True, stop=True,
                )
                # 1/F = exp(-ln F), both on ScalarE, off the PE chain
                lnf = spool.tile([DP, QBLK], f32, tag="lnf")
                nc.scalar.activation(lnf[:], ps_f[:], AF.Ln)
                rcf = spool.tile([DP, QBLK], f32, tag="rcf")
                nc.scalar.activation(rcf[:], lnf[:], AF.Exp, scale=-1.0)
                nc.vector.tensor_tensor(attn[:, h], ps_o[:], rcf[:], ALU.mult)
            if nxt is not None:
                qt_cur = qt_nxt

            if blk == 0 and b + 2 < B_LOC:
                xts[b + 2] = phase_x(b + 2)

            # output projection per 128-row q chunk
            for qc in range(QC_PER_BLK):
                cs = slice(qc * P, (qc + 1) * P)
                ps_m1 = psout.tile([P, 384], f32, tag="m1")
                ps_m2 = psout.tile([P, 256], f32, tag="m2")
                ost = opool.tile([P, E], f32, tag="ost")
                for h in range(H):
                    nc.tensor.matmul(
                        ps_m1[:], attn[:, h, cs], wo_s[:, h, 0:384],
                        start=(h == 0), stop=(h == H - 1),
                    )
                nc.vector.tensor_tensor(
                    ost[:, 0:384], ps_m1[:], bo_b[:, 0:384], ALU.add
                )
                for h in range(H):
                    nc.tensor.matmul(
                        ps_m2[:], attn[:, h, cs], wo_s[:, h, 384:640],
                        start=(h == 0), stop=(h == H - 1),
                    )
                nc.vector.tensor_tensor(
                    ost[:, 384:640], ps_m2[:], bo_b[:, 384:640], ALU.add
                )
                q0 = blk * QBLK + qc * P
                nc.sync.dma_start(out_d[b, q0 : q0 + P, :], ost[:])


    nc.compile()
    return nc


def _get_built():
    global _BUILT
    if _BUILT is None:
        _BUILT = _build()
    return _BUILT


def kernel(x, y, Wq, bq, Wk, bk, Wv, bv, Wo, bo):
    global LAST_RESULTS
    from concourse.bass_utils import run_bass_kernel_spmd

    nc = _get_built()

    x = np.ascontiguousarray(np.asarray(x, np.float32))
    y = np.ascontiguousarray(np.asarray(y, np.float32))
    wk_pad = _pad_cols(np.asarray(Wk, np.float32))
    bq_pad = _pad_vec(np.asarray(bq, np.float32) * SCALE)
    # per-key softmax bias: sbias[b,k,h] = k_raw[b,k,h,:] . bq_h  (the bk
    # term is per-query constant and cancels in softmax)
    wsb = (wk_pad * bq_pad[None, :]).reshape(C, H, DP).sum(-1)  # [C, H]
    sbias = np.einsum("bkc,ch->bkh", y, wsb)  # [B, SKV, H]
    shared = {
        "wq": np.ascontiguousarray(
            _pad_cols(np.asarray(Wq, np.float32) * SCALE)[:, _PERM]
        ).astype(BF16),
        "wk": np.ascontiguousarray(wk_pad[:, _PERM]),
        "wv": _pad_cols(np.asarray(Wv, np.float32)),
        "bv": np.broadcast_to(_pad_vec(np.asarray(bv, np.float32)), (P, EP)).copy(),
        "wo": _pad_rows(np.asarray(Wo, np.float32)),
        "bo": np.broadcast_to(np.asarray(bo, np.float32), (P, E)).copy(),
        "ones": np.ones((SKV, DP), np.float32),
        "ident": np.eye(P, dtype=np.float32),
        "identb": np.eye(P, dtype=np.float32).astype(BF16),
    }
    shared = {k: np.ascontiguousarray(v) for k, v in shared.items()}

    in_maps = []
    for core in range(N_CORES):
        bs = slice(core * B_LOC, (core + 1) * B_LOC)
        m = {
            "x": np.ascontiguousarray(x[bs]).astype(BF16),
            "y": np.ascontiguousarray(y[bs]),
            "sbias": np.ascontiguousarray(sbias[bs].transpose(1, 2, 0)),
        }
        m.update(shared)
        in_maps.append(m)

    res = run_bass_kernel_spmd(nc, in_maps, core_ids=list(range(N_CORES)))
    LAST_RESULTS = res

    out = np.empty((B, SQ, E), np.float32)
    for core in range(N_CORES):
        out[core * B_LOC : (core + 1) * B_LOC] = res.results[core]["out"]
    return out

